# revision 22
# baseline (speedup 1.0000x reference)
"""Trainium2 Bass kernel for nn_MemoryCell (scatter_memory).

Full-input contract: kernel(**inputs) takes the complete (unsharded) numpy
inputs and returns the full [NB*B, H] output.

Math (B == H == 1024, NB == 5, T == 128):
    enc  = features[:, 0, :]                         # [B, H] - only slice used
    h    = states.reshape(NB, H)
    gate = sigmoid(enc @ (h + keys).T)               # [B, NB]
    pre  = (h @ Uw.T + keys @ Vw.T)[:, None, :] + (enc @ Ww.T)[None, :, :]
    cand = where(pre >= 0, pre, prelu_a * pre)
    new[i, b, j] = h[i, j] + gate[j, i] * cand[i, b, j]   # B==H broadcast quirk
    out  = sign(new) with exact zeros -> +1, reshaped [NB*B, H]

Because the output is pure signs, the elementwise tail collapses to a
per-(i, j) THRESHOLD on ew = enc @ Ww.T:
    out[i, b, j] = +1  iff  ew[b, j] >= THR[i, j]
with THR = t_cand - huv, t_cand = (q >= 0 ? q : q / a_j), q = -h/gate
(valid for prelu_a > 0; PReLU is monotone there).  gate/huv/THR involve only
O(H*NB) work on tiny tensors and sit on the host (float64, exact), applied
during the gather/unshard step along with the sign expansion.

The device work per core is ONE [512, 1024] x [1024, 256] GEMM in plain fp16
(both operands round-to-nearest fp16: the PE upconverts fp16 to FP22
losslessly, so HW matches the host simulation; together with the fp16
round-trip of ew itself this measures 132 sign flips of 5.24M, rel err
0.010 vs the 0.02 gate) and ships ew back as fp16 (0.26 MB/core).

Sharding: 2 b-halves x 4 j-quarters = 8 cores.  Per core DMA: Ww quarter
(0.5 MB fp16) + enc half (1 MB fp16) in, ew (0.26 MB fp16) out.  Inputs
stream k-chunk-paced on BOTH HWDGE rings (sync + scalar) so the matmul
series chases the arrivals; a short identity warm-up keeps the PE HAM
activity window busy so the series runs at the warm clock.
"""

import os
import numpy as np

H = 1024
B = 1024
NB = 5
NCORES = 8
NJ = 4              # j-quarters of 256 columns
NBH = 2             # b-halves of 512 rows
BS = 256            # b sub-chunk (PSUM tile width)

_NC_CACHE = {}


def _build_nc():
    from concourse import bacc, mybir
    import concourse.tile as tile

    f32 = mybir.dt.float32
    f16 = mybir.dt.float16
    AF = mybir.ActivationFunctionType

    nc = bacc.Bacc("TRN2", debug=False, num_devices=NCORES)

    # per sub: [p, kc, col] with k = kc*128 + p
    encd = nc.dram_tensor("encd", [2, 128, 8, BS], f16, kind="ExternalInput").ap()
    # [khalf, p, kl, jt, j]
    wd = nc.dram_tensor("wd", [2, 128, 4, 2, 128], f16, kind="ExternalInput").ap()
    outd = nc.dram_tensor("out", [2, 128, 2, BS], f16, kind="ExternalOutput").ap()

    with tile.TileContext(nc) as tc:
        with (
            tc.tile_pool(name="res", bufs=1) as res,
            tc.tile_pool(name="work", bufs=1) as work,
            tc.tile_pool(name="pp", bufs=1, space="PSUM") as pp,
        ):
            # PE warm-up identity, built on-chip first (gpsimd owns
            # affine_select; its DMA issues follow right after)
            from concourse.masks import make_identity
            id_sb = res.tile([128, 128], f16, name="id_sb")
            make_identity(nc, id_sb)

            # ---- input DMAs, ALL on the sync HWDGE ring: a single ring
            # gets the full 16-engine SDMA capacity (same aggregate rate as
            # two rings) and avoids the scalar ring's late start behind its
            # ACT_TABLE_LOAD, which otherwise makes its last piece the
            # semaphore laggard.  Pieces alternate subs so the interleaved
            # matmul series is evenly paced. ----
            w_t = []
            for kh in range(2):
                w = res.tile([128, 4, 2, 128], f16, name=f"w{kh}", tag=f"w{kh}")
                nc.sync.dma_start(w, wd[kh])
                w_t.append(w)
            # e pieces per sub: k-chunk groups (0-2), (3-5), (6-7)
            e_t = [[None] * 3 for _ in range(2)]
            for pi, (kc0, nk) in enumerate([(0, 3), (3, 3), (6, 2)]):
                for s in range(2):
                    e = res.tile([128, nk, BS], f16, name=f"e{s}{pi}")
                    nc.sync.dma_start(e, encd[s][:, kc0:kc0 + nk, :])
                    e_t[s][pi] = e

            # keep the HAM activity window busy so the real series starts
            # at the warm clock
            psum_warm = pp.tile([128, 128], f32, name="psum_warm")
            for _ in range(28):
                nc.tensor.matmul(psum_warm, lhsT=id_sb, rhs=id_sb,
                                 start=True, stop=True)

            ps = [[pp.tile([128, BS], f32, name=f"ps{s}{t}") for t in range(2)]
                  for s in range(2)]
            ew_sb = [work.tile([128, 2, BS], f16, name=f"ew{s}")
                     for s in range(2)]

            # k-major, subs interleaved: the series chases piece arrivals on
            # both rings, so after the last piece's semaphore only the final
            # k-pair's matmuls remain
            for k in range(8):
                pi = min(k // 3, 2)
                kl = k - (0, 3, 6)[pi]
                for s in range(2):
                    for t in range(2):
                        nc.tensor.matmul(
                            ps[s][t], lhsT=w_t[k // 4][:, k % 4, t, :],
                            rhs=e_t[s][pi][:, kl, :],
                            start=(k == 0), stop=(k == 7))
            # psum -> fp16 SBUF on two engines, then ship each sub from its
            # own HWDGE ring so the two out-DMAs overlap
            for s in range(2):
                nc.scalar.activation(ew_sb[s][:, 0, :], ps[s][0], AF.Copy)
                nc.vector.tensor_copy(out=ew_sb[s][:, 1, :], in_=ps[s][1])
                (nc.sync if s == 0 else nc.scalar).dma_start(outd[s], ew_sb[s])

    nc.compile()
    return nc


def _get_nc():
    nc = _NC_CACHE.get("nc")
    if nc is None:
        nc = _build_nc()
        _NC_CACHE["nc"] = nc
    return nc


def _pack_enc(enc_half):
    # [512 b, 1024 k] f32 -> [2, 128, 8, 256] f16: [s, p, kc, col]
    e = np.ascontiguousarray(enc_half.T).astype(np.float16)   # [k, b]
    e = e.reshape(8, 128, 2, BS)              # [kc, p, s, col]
    return np.ascontiguousarray(e.transpose(2, 1, 0, 3))


def _pack_w(Wq):
    # [256 j, 1024 k] f32 -> [2, 128, 4, 2, 128] f16
    w = np.ascontiguousarray(Wq.T).astype(np.float16)         # [k, j]
    w = w.reshape(2, 4, 128, 2, 128)          # [kh, kl, p, jt, j]
    return np.ascontiguousarray(w.transpose(0, 2, 1, 3, 4))


def _host_fallback(enc, h, keys, Uw, Vw, Ww, prelu_a):
    # exact reference math (only used if prelu_a has non-positive entries,
    # where the threshold fold is invalid; never hit for the spec'd inputs)
    gate = 1.0 / (1.0 + np.exp(-(enc @ (h + keys).T)))
    pre = (h @ Uw.T + keys @ Vw.T)[:, None, :] + (enc @ Ww.T)[None, :, :]
    cand = np.where(pre >= 0, pre, prelu_a * pre)
    new = h[:, None, :] + gate.T[:, None, :] * cand
    new = np.where(new == 0, 0.1, new)
    return np.where(new >= 0, np.float32(1.0), np.float32(-1.0)).reshape(
        NB * B, H).astype(np.float32)


def kernel(features, states, Uw, Vw, Ww, keys, prelu_a):
    from concourse import bass_utils

    features = np.asarray(features)
    states = np.asarray(states, dtype=np.float32)
    Uw = np.asarray(Uw, dtype=np.float32)
    Vw = np.asarray(Vw, dtype=np.float32)
    Ww = np.asarray(Ww, dtype=np.float32)
    keys = np.asarray(keys, dtype=np.float32)
    prelu_a = np.asarray(prelu_a, dtype=np.float32)

    enc = np.ascontiguousarray(features[:, 0, :], dtype=np.float32)  # [B, H]
    h = states.reshape(NB, H)

    if np.any(prelu_a <= 0):
        return _host_fallback(enc.astype(np.float64), h.astype(np.float64),
                              keys.astype(np.float64), Uw.astype(np.float64),
                              Vw.astype(np.float64), Ww.astype(np.float64),
                              prelu_a.astype(np.float64))

    # ---- tiny tensors -> per-(i, j) thresholds, in float64 ----
    enc64 = enc.astype(np.float64)
    h64 = h.astype(np.float64)
    k64 = keys.astype(np.float64)
    gateT = 1.0 / (1.0 + np.exp(-(enc64 @ (h64 + k64).T))).T      # [i, j]
    huv = h64 @ Uw.astype(np.float64).T + k64 @ Vw.astype(np.float64).T
    q = -h64 / gateT
    a = prelu_a.astype(np.float64)[None, :]
    t_cand = np.where(q >= 0, q, q / a)
    THR = np.clip(t_cand - huv, -1e30, 1e30).astype(np.float32)   # [i, j]

    nc = _get_nc()

    enc_packs = [_pack_enc(enc[bh * 512:(bh + 1) * 512]) for bh in range(NBH)]
    w_packs = [_pack_w(Ww[jq * 256:(jq + 1) * 256]) for jq in range(NJ)]
    in_maps = []
    for c in range(NCORES):
        jq, bh = divmod(c, 2)
        in_maps.append({
            "encd": enc_packs[bh],
            "wd": w_packs[jq],
        })

    trace = bool(int(os.environ.get("KERNEL_TRACE", "0")))
    res = bass_utils.run_bass_kernel_spmd(
        nc, in_maps, core_ids=list(range(NCORES)), trace=trace)
    kernel.last_result = res

    # gather ew [b, j] from the cores, then apply the thresholds
    ew = np.empty((B, H), dtype=np.float32)
    ev = ew.reshape(NBH, 2, BS, NJ, 2, 128)    # [bh, s, col, jq, t, p]
    for c in range(NCORES):
        jq, bh = divmod(c, 2)
        o = res.results[c]["out"]              # [s, p, t, col] f16
        ev[bh, :, :, jq, :, :] = o.transpose(0, 3, 2, 1)  # [s, col, t, p]
    one = np.float32(1.0)
    neg = np.float32(-1.0)
    full = np.where(ew[None, :, :] >= THR[:, None, :], one, neg)
    return np.ascontiguousarray(full.reshape(NB * B, H), dtype=np.float32)


# revision 27
# speedup vs baseline: 1.1789x; 1.1789x over previous
"""Trainium2 Bass kernel for nn_MemoryCell (scatter_memory).

Full-input contract: kernel(**inputs) takes the complete (unsharded) numpy
inputs and returns the full [NB*B, H] output.

Math (B == H == 1024, NB == 5, T == 128):
    enc  = features[:, 0, :]                         # [B, H] - only slice used
    h    = states.reshape(NB, H)
    gate = sigmoid(enc @ (h + keys).T)               # [B, NB]
    pre  = (h @ Uw.T + keys @ Vw.T)[:, None, :] + (enc @ Ww.T)[None, :, :]
    cand = where(pre >= 0, pre, prelu_a * pre)
    new[i, b, j] = h[i, j] + gate[j, i] * cand[i, b, j]   # B==H broadcast quirk
    out  = sign(new) with exact zeros -> +1, reshaped [NB*B, H]

Because the output is pure signs, the elementwise tail collapses to a
per-(i, j) THRESHOLD on ew = enc @ Ww.T:
    out[i, b, j] = +1  iff  ew[b, j] >= THR[i, j]
with THR = t_cand - huv, t_cand = (q >= 0 ? q : q / a_j), q = -h/gate
(valid for prelu_a > 0; PReLU is monotone there).  gate/huv/THR involve only
O(H*NB) work on tiny tensors and sit on the host (float64, exact), applied
during the gather/unshard step along with the sign expansion.

The device work per core is ONE [512, 1024] x [1024, 256] GEMM in plain fp16
(both operands round-to-nearest fp16: the PE upconverts fp16 to FP22
losslessly, so HW matches the host simulation; together with the fp16
round-trip of ew itself this measures 132 sign flips of 5.24M, rel err
0.010 vs the 0.02 gate) and ships ew back as fp16 (0.26 MB/core).

Sharding: 2 b-halves x 4 j-quarters = 8 cores.  Per core DMA: Ww quarter
(0.5 MB fp16) + enc half (1 MB fp16) in, ew (0.26 MB fp16) out.  Inputs
stream k-chunk-paced on BOTH HWDGE rings (sync + scalar) so the matmul
series chases the arrivals; a short identity warm-up keeps the PE HAM
activity window busy so the series runs at the warm clock.
"""

import os
import numpy as np

H = 1024
B = 1024
NB = 5
NCORES = 8
NJ = 4              # j-quarters of 256 columns
NBH = 2             # b-halves of 512 rows
BS = 256            # b sub-chunk (PSUM tile width)

_NC_CACHE = {}


def _build_nc():
    from concourse import bacc, mybir
    import concourse.tile as tile

    f32 = mybir.dt.float32
    f16 = mybir.dt.float16
    AF = mybir.ActivationFunctionType

    nc = bacc.Bacc("TRN2", debug=False, num_devices=NCORES)

    # g = sub*4 + kq; [p, kl, col] with k = (kq*2 + kl)*128 + p
    encd = nc.dram_tensor("encd", [8, 128, 2, BS], f16, kind="ExternalInput").ap()
    # [khalf, p, kl, jt, j]
    wd = nc.dram_tensor("wd", [2, 128, 4, 2, 128], f16, kind="ExternalInput").ap()
    outd = nc.dram_tensor("out", [2, 128, 2, BS], f16, kind="ExternalOutput").ap()

    with tile.TileContext(nc) as tc:
        with (
            tc.tile_pool(name="res", bufs=1) as res,
            tc.tile_pool(name="work", bufs=1) as work,
            tc.tile_pool(name="pp", bufs=1, space="PSUM") as pp,
        ):
            # PE warm-up identity, built on-chip first (gpsimd owns
            # affine_select; its DMA issues follow right after)
            from concourse.masks import make_identity
            id_sb = res.tile([128, 128], f16, name="id_sb")
            make_identity(nc, id_sb)

            # ---- input DMAs on both HWDGE rings.  Both weight halves go
            # on the sync ring: the scalar ring starts ~0.7us late (behind
            # its fixed ACT_TABLE_LOAD), so it carries only the four s1 enc
            # pieces and drains earlier instead of gating the last
            # semaphore. ----
            w_t = []
            for kh in range(2):
                w = res.tile([128, 4, 2, 128], f16, name=f"w{kh}", tag=f"w{kh}")
                nc.sync.dma_start(w, wd[kh])
                w_t.append(w)
            e_t = []
            for g in range(8):
                e = res.tile([128, 2, BS], f16, name=f"e{g}", tag=f"e{g}")
                # sync ring: s0 pieces; scalar ring: s1 pieces
                (nc.sync if g < 4 else nc.scalar).dma_start(e, encd[g])
                e_t.append(e)

            # keep the HAM activity window busy so the real series starts
            # at the warm clock
            psum_warm = pp.tile([128, 128], f32, name="psum_warm")
            for _ in range(28):
                nc.tensor.matmul(psum_warm, lhsT=id_sb, rhs=id_sb,
                                 start=True, stop=True)

            ps = [[pp.tile([128, BS], f32, name=f"ps{s}{t}") for t in range(2)]
                  for s in range(2)]
            ew_sb = [work.tile([128, 2, BS], f16, name=f"ew{s}")
                     for s in range(2)]

            # k-major, subs interleaved: the series chases piece arrivals on
            # both rings, so after the last piece's semaphore only the final
            # k-pair's matmuls remain
            for k in range(8):
                kq, kl = divmod(k, 2)
                for s in range(2):
                    for t in range(2):
                        nc.tensor.matmul(
                            ps[s][t], lhsT=w_t[k // 4][:, k % 4, t, :],
                            rhs=e_t[s * 4 + kq][:, kl, :],
                            start=(k == 0), stop=(k == 7))
            # psum -> fp16 SBUF on two engines, then ship each sub from its
            # own HWDGE ring so the two out-DMAs overlap
            for s in range(2):
                nc.scalar.activation(ew_sb[s][:, 0, :], ps[s][0], AF.Copy)
                nc.vector.tensor_copy(out=ew_sb[s][:, 1, :], in_=ps[s][1])
                (nc.sync if s == 0 else nc.scalar).dma_start(outd[s], ew_sb[s])

    nc.compile()
    return nc


def _get_nc():
    nc = _NC_CACHE.get("nc")
    if nc is None:
        nc = _build_nc()
        _NC_CACHE["nc"] = nc
    return nc


def _pack_enc(enc_half):
    # [512 b, 1024 k] f32 -> [8, 128, 2, 256] f16, g = sub*4 + kq
    e = np.ascontiguousarray(enc_half.T).astype(np.float16)   # [k, b]
    e = e.reshape(4, 2, 128, 2, BS)           # [kq, kl, p, s, col]
    e = e.transpose(3, 0, 2, 1, 4)            # [s, kq, p, kl, col]
    return np.ascontiguousarray(e.reshape(8, 128, 2, BS))


def _pack_w(Wq):
    # [256 j, 1024 k] f32 -> [2, 128, 4, 2, 128] f16
    w = np.ascontiguousarray(Wq.T).astype(np.float16)         # [k, j]
    w = w.reshape(2, 4, 128, 2, 128)          # [kh, kl, p, jt, j]
    return np.ascontiguousarray(w.transpose(0, 2, 1, 3, 4))


def _host_fallback(enc, h, keys, Uw, Vw, Ww, prelu_a):
    # exact reference math (only used if prelu_a has non-positive entries,
    # where the threshold fold is invalid; never hit for the spec'd inputs)
    gate = 1.0 / (1.0 + np.exp(-(enc @ (h + keys).T)))
    pre = (h @ Uw.T + keys @ Vw.T)[:, None, :] + (enc @ Ww.T)[None, :, :]
    cand = np.where(pre >= 0, pre, prelu_a * pre)
    new = h[:, None, :] + gate.T[:, None, :] * cand
    new = np.where(new == 0, 0.1, new)
    return np.where(new >= 0, np.float32(1.0), np.float32(-1.0)).reshape(
        NB * B, H).astype(np.float32)


def kernel(features, states, Uw, Vw, Ww, keys, prelu_a):
    from concourse import bass_utils

    features = np.asarray(features)
    states = np.asarray(states, dtype=np.float32)
    Uw = np.asarray(Uw, dtype=np.float32)
    Vw = np.asarray(Vw, dtype=np.float32)
    Ww = np.asarray(Ww, dtype=np.float32)
    keys = np.asarray(keys, dtype=np.float32)
    prelu_a = np.asarray(prelu_a, dtype=np.float32)

    enc = np.ascontiguousarray(features[:, 0, :], dtype=np.float32)  # [B, H]
    h = states.reshape(NB, H)

    if np.any(prelu_a <= 0):
        return _host_fallback(enc.astype(np.float64), h.astype(np.float64),
                              keys.astype(np.float64), Uw.astype(np.float64),
                              Vw.astype(np.float64), Ww.astype(np.float64),
                              prelu_a.astype(np.float64))

    # ---- tiny tensors -> per-(i, j) thresholds, in float64 ----
    enc64 = enc.astype(np.float64)
    h64 = h.astype(np.float64)
    k64 = keys.astype(np.float64)
    gateT = 1.0 / (1.0 + np.exp(-(enc64 @ (h64 + k64).T))).T      # [i, j]
    huv = h64 @ Uw.astype(np.float64).T + k64 @ Vw.astype(np.float64).T
    q = -h64 / gateT
    a = prelu_a.astype(np.float64)[None, :]
    t_cand = np.where(q >= 0, q, q / a)
    THR = np.clip(t_cand - huv, -1e30, 1e30).astype(np.float32)   # [i, j]

    nc = _get_nc()

    enc_packs = [_pack_enc(enc[bh * 512:(bh + 1) * 512]) for bh in range(NBH)]
    w_packs = [_pack_w(Ww[jq * 256:(jq + 1) * 256]) for jq in range(NJ)]
    in_maps = []
    for c in range(NCORES):
        jq, bh = divmod(c, 2)
        in_maps.append({
            "encd": enc_packs[bh],
            "wd": w_packs[jq],
        })

    trace = bool(int(os.environ.get("KERNEL_TRACE", "0")))
    res = bass_utils.run_bass_kernel_spmd(
        nc, in_maps, core_ids=list(range(NCORES)), trace=trace)
    kernel.last_result = res

    # gather ew [b, j] from the cores, then apply the thresholds
    ew = np.empty((B, H), dtype=np.float32)
    ev = ew.reshape(NBH, 2, BS, NJ, 2, 128)    # [bh, s, col, jq, t, p]
    for c in range(NCORES):
        jq, bh = divmod(c, 2)
        o = res.results[c]["out"]              # [s, p, t, col] f16
        ev[bh, :, :, jq, :, :] = o.transpose(0, 3, 2, 1)  # [s, col, t, p]
    one = np.float32(1.0)
    neg = np.float32(-1.0)
    full = np.where(ew[None, :, :] >= THR[:, None, :], one, neg)
    return np.ascontiguousarray(full.reshape(NB * B, H), dtype=np.float32)
